# revision 44
# baseline (speedup 1.0000x reference)
"""Masked multi-head attention on 8 Trainium2 NeuronCores (fp8 DoubleRow).

Sharding: core c = (b, hg) with b = c // 4, hg = c % 4. Each core computes the
full attention block for batch b restricted to heads [4*hg, 4*hg+4), including
its slice of the QKV projection and of the output projection. The host sums the
4 tensor-parallel partial outputs per batch (dividing out the fp8 weight
pre-scale) and adds the output bias (which also absorbs the V bias term).

Shapes are hardcoded for B=2, T=2048, D=1024, H=16, Hd=64.

Precision plan (validated against the reference in numpy emulation):
- QK projection: x and Wqk in e4m3 (x16 weight pre-scale), DoubleRow over
  dt-chunk pairs -> 4x PE throughput. Q/K fp8 noise largely washes out in
  softmax normalization.
- V projection: both sides residual-split (hi+lo e4m3), 3 DoubleRow terms
  (hi*hi + hi*lo + lo*hi) -> 1.33x, near-bf16 accuracy.
- Scores: kqt holds (K|0) in the two DoubleRow weight slots; moving is the
  broadcast (Q|Q) -> per-head score tile in half the cycles.
- exp on Act with scale=1/256 folding out the x16 Q and K pre-scales; output
  e4m3 for middle query blocks, bf16 for block 0 (few-key softmax rows where
  fp8 P noise is largest) and block 3 (PE/Act balance).
- AV: query blocks 1-2 use DoubleRow (Vhi|Vlo)(P|P) -> V-side exact, P single
  e4m3; blocks 0 and 3 run bf16 AV with a bf16 V copy. Softmax denominators
  ride a 65th ones-column (hi=1, lo=0).
- Out-projection: bf16 throughout (splitting O costs more elementwise work
  than the PE it saves).

Schedule follows the bf16 baseline: host pre-transposes x (zero PE
transposes), biases fused into PSUM evacuations, denominators via the ones
column, normalize deferred into the next pair's filler stream with a
Pool-issued partition broadcast, scores running k-tiles ahead of AV, and a
cost-weighted filler queue (QKV/V/out-proj work) keeping the PE continuously
busy so its p-state never drops. Block 0's Q/K accumulates 4-ways in parallel
behind a PE warmup chain; block-0 PSUM evacuations ride the Act engine while
it is still idle. The tail staggers output staging across Act + DVE and
drains the last DMAs through both the Act HWDGE and Pool SWDGE queues.
"""

import numpy as np
from contextlib import ExitStack
from collections import deque

import ml_dtypes

import concourse.bass as bass
import concourse.bacc as bacc
import concourse.mybir as mybir
import concourse.tile as tile
from concourse.bass_utils import run_bass_kernel_spmd

B, T, D = 2, 2048, 1024
H, HD = 16, 64
HL = 4               # heads per core
NCORES = 8
TQ = 512             # query tile (matmul moving free dim)
TK = 128             # key tile
NQT = T // TQ        # 4
NKT = T // TK        # 16
NDT = D // 128       # 8

F32 = mybir.dt.float32
BF16 = mybir.dt.bfloat16
E4M3 = mybir.dt.float8e4
DR = mybir.MatmulPerfMode.DoubleRow
EXP = mybir.ActivationFunctionType.Exp
COPY = mybir.ActivationFunctionType.Copy
IDENT = mybir.ActivationFunctionType.Identity
MULT = mybir.AluOpType.mult
ADD = mybir.AluOpType.add
SUB = mybir.AluOpType.subtract

WS = 16.0            # fp8 weight pre-scale
ESC = 1.0 / (WS * WS)  # exp reads scores scaled by 16(Q) * 16(K)

LAST_RESULTS = None  # BassKernelResults of the most recent run (for test.py)

import os as _os
_B0 = float(_os.environ.get("KBONUS0", "0"))
_B1 = float(_os.environ.get("KBONUS1", "0"))
_LA = int(_os.environ.get("KLOOKAHEAD", "3"))
_PACE = float(_os.environ.get("KPACE", "0.85"))
_PTB = int(_os.environ.get("KPTBUFS", "12"))

# query blocks whose AV runs bf16 (P stays bf16 there)
BF16_AV_QI = (0, 3)


def _build_mha(tc, out_ap, in_aps):
    nc = tc.nc
    xh_d = in_aps["xh"]        # [D, T]  e4m3  fp8(x^T)
    xl_d = in_aps["xl"]        # [D, T]  e4m3  fp8(x^T - xh)
    wqk_d = in_aps["wqk"]      # [D, 512] e4m3 (16*(Qh0..3|Kh0..3), Q pre-scaled)
    bqk_d = in_aps["bqk"]      # [128, 4] f32 (16x biases, col fb per-partition)
    wvh_d = in_aps["wvh"]      # [D, 256] e4m3 hi of 16*Wv
    wvl_d = in_aps["wvl"]      # [D, 256] e4m3 lo
    wout_d = in_aps["wout"]    # [128, 2, 1024] bf16
    m8_d = in_aps["mask8"]     # [128, 896] e4m3
    mb_d = in_aps["maskb"]     # [128, 896] bf16

    with ExitStack() as ctx:
        ctx.enter_context(nc.allow_low_precision(reason="fp8 matmul pipeline"))
        const = ctx.enter_context(tc.tile_pool(name="const", bufs=1))
        big = ctx.enter_context(tc.tile_pool(name="big", bufs=1))

        # Persistent activations.
        # qqt[:, p, t]: Q^T (x16, e4m3); rows 0:64 even head of pair p,
        # 64:128 odd head.
        qqt = big.tile([128, 2, T], E4M3)
        # kqt[:, p, s, t]: s=0 K^T (x16, e4m3), s=1 zeros (DoubleRow dummy)
        kqt = big.tile([128, 2, 2, T], E4M3)
        # vp[:, kt, s, h, 0:64] = hi/lo of 16*V; [..., 64] ones column
        # (hi=1, lo=0). Only k-tiles 0..11 (blocks 1-2 consume them).
        # head slot padded to 80 cols: the dual-fp8 LDWEIGHTS requires the
        # hi/lo slot stride (4*80=320) to be 64-aligned
        vp = big.tile([128, 12, 2, HL, 80], E4M3)
        # vpb: bf16 16*V for the bf16-AV blocks (all 16 k-tiles)
        vpb = big.tile([128, NKT, HL, 65], BF16)
        # ot[p][:, t]: normalized attention output^T (x16), bf16
        ot = [big.tile([128, T], BF16, name=f"ot{p}") for p in range(2)]

        with ExitStack() as pctx:
            # PSUM: sp(2 banks x2) + pq(1 x2) + av0 + av1 = 8 banks
            ps = pctx.enter_context(tc.tile_pool(name="ps", bufs=2,
                                                 space="PSUM"))
            ps_av = pctx.enter_context(tc.tile_pool(name="ps_av", bufs=1,
                                                    space="PSUM"))
            wpool = pctx.enter_context(tc.tile_pool(name="w", bufs=1))
            xt_p = pctx.enter_context(tc.tile_pool(name="xt", bufs=4))
            pt_p = pctx.enter_context(tc.tile_pool(name="ptile", bufs=_PTB))
            nrm_p = pctx.enter_context(tc.tile_pool(name="nrm", bufs=8))

            # PE p-state warmup: tiny matmuls keep the tensor engine busy
            # through its clock ramp while the first DMAs land.
            wsrc = const.tile([1, 64], BF16)
            nc.gpsimd.memset(wsrc[:], 1.0)
            # preload the exp activation table while Act is idle (1.3us)
            wjunk = const.tile([1, 64], BF16)
            nc.scalar.activation(wjunk[:], wsrc[:], EXP)
            warm = ps.tile([1, 64], F32, tag="pq")
            for _ in range(64):
                nc.tensor.matmul(warm[:], wsrc[0:1, 0:1], wsrc[:],
                                 start=True, stop=True)

            # ones columns + the DoubleRow zero slot, built in-place
            nc.gpsimd.memset(vp[:, :, 0, :, 64], 1.0)
            nc.gpsimd.memset(vp[:, :, 1, :, 64], 0.0)
            nc.gpsimd.memset(vpb[:, :, :, 64], 1.0)
            nc.gpsimd.memset(kqt[:, :, 1, :], 0.0)

            xhs = {}
            xls = {}

            def emit_xh_dma(tt):
                xh = xt_p.tile([128, NDT, TQ], E4M3, tag="xh", name=f"xh_{tt}")
                src = xh_d[:, tt * TQ:(tt + 1) * TQ].rearrange(
                    "(o p) t -> p o t", p=128)
                nc.sync.dma_start(xh[:], src)
                xhs[tt] = xh

            def emit_xl_dma(tt, eng=None):
                xl = xt_p.tile([128, NDT, TQ], E4M3, tag="xl", name=f"xl_{tt}")
                src = xl_d[:, tt * TQ:(tt + 1) * TQ].rearrange(
                    "(o p) t -> p o t", p=128)
                (eng or nc.gpsimd).dma_start(xl[:], src)
                xls[tt] = xl

            # DMA order = first-use order. wqk/xh0 stream in dt-quad chunks
            # so the 4-way parallel QK(0) accumulation can start on the first
            # chunk; side inputs ride the DVE HWDGE queue so they don't wait
            # behind the critical wqk/xh0 stream.
            wqk = wpool.tile([128, NDT, 512], E4M3)
            wqk_src = wqk_d.rearrange("(o p) f -> p o f", p=128)
            xh0 = xt_p.tile([128, NDT, TQ], E4M3, tag="xh", name="xh_0")
            xh0_src = xh_d[:, 0:TQ].rearrange("(o p) t -> p o t", p=128)
            xhs[0] = xh0
            # pair-0's K/Q weight columns ride the first DMA so its score
            # pipeline (and the Act exp stream) starts as early as possible
            for dh in range(2):
                s = slice(4 * dh, 4 * dh + 4)
                nc.sync.dma_start(wqk[:, s, :], wqk_src[:, s, :])
                nc.sync.dma_start(xh0[:, s, :], xh0_src[:, s, :])
            bqk = const.tile([128, 4], F32)
            nc.sync.dma_start(bqk[:], bqk_d)
            maskb = const.tile([128, 896], BF16)
            nc.sync.dma_start(maskb[:], mb_d)
            wvh = wpool.tile([128, NDT, 256], E4M3)
            nc.sync.dma_start(wvh[:], wvh_d.rearrange("(o p) f -> p o f",
                                                        p=128))
            wvl = wpool.tile([128, NDT, 256], E4M3)
            nc.sync.dma_start(wvl[:], wvl_d.rearrange("(o p) f -> p o f",
                                                        p=128))
            emit_xl_dma(0, nc.sync)
            emit_xh_dma(1)
            mask8 = const.tile([128, 896], E4M3)
            nc.gpsimd.dma_start(mask8[:], m8_d)
            wout = const.tile([128, 2, 1024], BF16)
            nc.gpsimd.dma_start(wout[:], wout_d)
            emit_xh_dma(2)
            emit_xl_dma(1)
            emit_xh_dma(3)
            emit_xl_dma(2)
            emit_xl_dma(3)

            def qk_matmuls(pq, fb, xh, skip=False):
                for j in range(4):
                    s = slice(2 * j, 2 * j + 2)
                    nc.tensor.matmul(pq[:], wqk[:, s, fb * 128:(fb + 1) * 128],
                                     xh[:, s, :], start=(j == 0),
                                     stop=(j == 3), perf_mode=DR,
                                     skip_group_check=skip)

            def evac_fb(tt, fb, pq, eng):
                """PSUM -> qqt/kqt with the (x16) bias fused; eng = DVE or Act
                (Act only while it is still idle, pre-exp)."""
                cols = slice(tt * TQ, (tt + 1) * TQ)
                dst = (qqt[:, fb, cols] if fb < 2
                       else kqt[:, fb - 2, 0, cols])
                if eng == "act":
                    nc.scalar.activation(dst, pq[:], IDENT,
                                         bias=bqk[:, fb:fb + 1])
                else:
                    nc.vector.tensor_scalar(out=dst, in0=pq[:],
                                            scalar1=bqk[:, fb:fb + 1],
                                            scalar2=None, op0=ADD)

            def emit_fb(tt, fb):
                """One 128-col block of Q^T/K^T via fp8 DoubleRow. Q
                evacuations ride Act (their consumers stall on the DVE
                queue at block starts); K evacuations stay on DVE."""
                pq = ps.tile([128, TQ], F32, tag="pq", name=f"pq_{tt}_{fb}")
                qk_matmuls(pq, fb, xhs[tt])
                evac_fb(tt, fb, pq, "dve")

            def emit_v(tt, ts):
                """V rows for one 128-token tile: both-split fp8 DoubleRow
                (hi*hi + hi*lo + lo*hi), evacuated to vp hi/lo + bf16 vpb."""
                xh, xl = xhs[tt], xls[tt]
                kt = tt * 4 + ts
                pv = ps.tile([128, 512], F32, tag="pq", name=f"pv_{tt}_{ts}")
                tok = slice(ts * 128, (ts + 1) * 128)
                first = True
                for wv, xx in ((wvh, xh), (wvl, xh), (wvh, xl)):
                    for j in range(4):
                        s = slice(2 * j, 2 * j + 2)
                        nc.tensor.matmul(pv[:, 0:256], xx[:, s, tok],
                                         wv[:, s, :], start=first,
                                         stop=(wv is wvh and xx is xl
                                               and j == 3), perf_mode=DR)
                        first = False
                src = pv[:, 0:256].rearrange("p (h e) -> p h e", e=HD)
                if kt < 12:
                    nc.vector.tensor_copy(vp[:, kt, 0, :, 0:64], src)
                    nc.vector.scalar_tensor_tensor(
                        vp[:, kt, 1, :, 0:64], src, 0.0,
                        vp[:, kt, 0, :, 0:64], ADD, SUB)
                if kt < 4 or kt >= 12:
                    # needed early (block 0) or without a vp slot (kt>=12):
                    # direct PSUM copy on DVE
                    nc.vector.tensor_copy(vpb[:, kt, :, 0:64], src)
                else:
                    # rebuild bf16 V from the fp8 hi/lo on the idle Pool
                    # engine, keeping the DVE queue short
                    nc.gpsimd.tensor_tensor(vpb[:, kt, :, 0:64],
                                            vp[:, kt, 0, :, 0:64],
                                            vp[:, kt, 1, :, 0:64], ADD)

            # (PE-ns cost, closure, deadline) triples for the filler queue.
            # deadline = (qi, kt) before which the unit MUST have emitted
            # (tile-framework program order): budget pacing may defer units,
            # the deadline pop may not.
            def q_units(tt):
                return [(427, (lambda tt=tt, fb=fb: emit_fb(tt, fb)),
                         (tt, -1)) for fb in range(2)]

            def k_units(tt):
                return [(427, (lambda tt=tt, fb=fb: emit_fb(tt, fb)),
                         (tt, max(0, 4 * tt - 4))) for fb in range(2, 4)]

            def v_units(tt):
                return [(640, (lambda tt=tt, ts=ts: emit_v(tt, ts)),
                         (tt, 4 * tt + ts)) for ts in range(4)]

            def pop_due(fillers, qi, kt):
                due = [u for u in fillers if u[2] is not None
                       and u[2] <= (qi, kt)]
                if due:
                    rest = [u for u in fillers if u[2] is None
                            or u[2] > (qi, kt)]
                    fillers.clear()
                    fillers.extend(rest)
                    for _, f, _d in due:
                        f()

            def emit_scores(p, qi, kt):
                """Score tile for both heads of pair p via fp8 DoubleRow with
                the (K|0) weight slots and broadcast (Q|Q) moving; exp + mask.
                bf16 pt for the bf16-AV blocks, e4m3 otherwise."""
                lo = 128 * (kt - 4 * qi)     # causal start, local cols
                c_lo = max(lo, 0)
                w = TQ
                bavq = qi in BF16_AV_QI
                sp = ps.tile([128, 2, TQ], F32, tag="sp",
                             name=f"sp_{p}_{qi}_{kt}")
                for a in range(2):
                    rows = slice(64 * a, 64 * a + 64)
                    mv = qqt[rows, p,
                             qi * TQ + c_lo:(qi + 1) * TQ][:, None, :]
                    nc.tensor.matmul(
                        sp[:, a, c_lo:w],
                        kqt[rows, p, :, kt * TK:(kt + 1) * TK],
                        mv.broadcast_to([64, 2, w - c_lo]),
                        start=True, stop=True, perf_mode=DR)
                pt = pt_p.tile([128, 2, TQ], BF16 if bavq else E4M3,
                               tag=("ptb" if bavq else "pt8"),
                               name=f"pt_{p}_{qi}_{kt}")
                nc.scalar.activation(pt[:, :, c_lo:w], sp[:, :, c_lo:w], EXP,
                                     scale=ESC)
                if -128 < lo < w:
                    c0 = 384 - 128 * (kt - 4 * qi)
                    m_lo, m_hi = c_lo, min(lo + 128, w)
                    msk = maskb if bavq else mask8
                    nc.gpsimd.tensor_tensor(
                        pt[:, :, m_lo:m_hi], pt[:, :, m_lo:m_hi],
                        msk[:, None, c0 + m_lo:c0 + m_hi].broadcast_to(
                            [128, 2, m_hi - m_lo]), MULT)
                return pt, c_lo

            def emit_c(p, qi, fillers, defer=True, pre_sc=None):
                """AV block for pair p. Returns deferred normalize units."""
                w = TQ
                bavq = qi in BF16_AV_QI
                av0 = ps_av.tile([128, TQ], F32, tag="av0",
                                 name=f"av0_{p}_{qi}")
                av1 = ps_av.tile([128, TQ], F32, tag="av1",
                                 name=f"av1_{p}_{qi}")
                nkt = 4 * qi + 4
                pop_due(fillers, qi, -1)
                sc = dict(pre_sc) if pre_sc else {0: emit_scores(p, qi, 0)}
                # flush the previous pair's deferred normalize first (their
                # multiplies release the single-buffered av banks), plus
                # filler to cover the first exp's latency; pair-1 starts also
                # force the next block's Q/K projections through
                for _ in range(min(2, len(fillers))):
                    fillers.popleft()[1]()
                for _k in (1, 2):
                    if _k < nkt and _k not in sc:
                        sc[_k] = emit_scores(p, qi, _k)
                debt = 0.0
                for kt in range(nkt):
                    if kt + _LA < nkt and kt + _LA not in sc:
                        sc[kt + _LA] = emit_scores(p, qi, kt + _LA)
                    # pace filler popping against this k-tile's Act slack:
                    # exp time minus the PE's own score+AV time, so fillers
                    # never crowd out the scores Act is waiting for
                    wc = w - max(128 * (kt - 4 * qi), 0)
                    act_step = (1.667 * wc + 175.0) * _PACE
                    pe_step = 0.4167 * wc * (3.0 if bavq else 2.0)
                    debt += max(act_step - pe_step, 0.0) + (_B0, _B1,
                                                           0.0, 0.0)[qi]
                    pop_due(fillers, qi, kt)
                    while fillers and debt >= fillers[0][0]:
                        c, f, _d = fillers.popleft()
                        f()
                        debt -= c
                    if kt not in sc:
                        sc[kt] = emit_scores(p, qi, kt)
                    pt, c_lo = sc.pop(kt)
                    # av rows 0:64 = O^T (x16), row 64 = softmax denominator
                    for a, av in ((0, av0), (1, av1)):
                        if bavq:
                            nc.tensor.matmul(
                                av[0:65, c_lo:w], vpb[:, kt, 2 * p + a, :],
                                pt[:, a, c_lo:w],
                                start=(kt == 0), stop=(kt == nkt - 1),
                                skip_group_check=True)
                        else:
                            mv = pt[:, a, c_lo:w][:, None, :]
                            nc.tensor.matmul(
                                av[0:65, c_lo:w],
                                vp[:, kt, :, 2 * p + a, 0:65],
                                mv.broadcast_to([128, 2, w - c_lo]),
                                start=(kt == 0), stop=(kt == nkt - 1),
                                perf_mode=DR, skip_group_check=True)
                srcs = [av0, av1]
                if not defer:
                    # last pair: normalize in 128-col pieces so the tail
                    # out-projections pipeline with the normalize chain
                    # (the caller emits po(ts) right after piece ts-12)
                    rt = nrm_p.tile([1, 2, TQ], BF16, tag="rec",
                                    name="rect")
                    bct = [nrm_p.tile([64, TQ], BF16, tag=f"bc{a}",
                                      name=f"bct_{a}") for a in range(2)]

                    def norm_piece(j):
                        cs = slice(j * 128, (j + 1) * 128)
                        ocs = slice(qi * TQ + j * 128, qi * TQ + (j + 1) * 128)
                        for a in range(2):
                            nc.vector.reciprocal(rt[0:1, a, cs],
                                                 srcs[a][64:65, cs])
                            nc.gpsimd.partition_broadcast(bct[a][0:64, cs],
                                                          rt[0:1, a, cs])
                            nc.vector.tensor_tensor(
                                ot[p][64 * a:64 * a + 64, ocs],
                                srcs[a][0:64, cs], bct[a][0:64, cs], MULT)
                    return norm_piece
                # Both reciprocals land on partition 0 of one tile; the
                # multiplies are deferred into the next pair's filler stream.
                rec2 = nrm_p.tile([1, 2, TQ], BF16, tag="rec",
                                  name=f"rec_{p}_{qi}")
                nc.vector.reciprocal(rec2[0:1, 0, :], srcs[0][64:65, :])
                nc.vector.reciprocal(rec2[0:1, 1, :], srcs[1][64:65, :])
                bcs = []
                for a in range(2):
                    bc = nrm_p.tile([64, TQ], BF16, tag=f"bc{a}",
                                    name=f"bc_{p}_{qi}_{a}")
                    nc.gpsimd.partition_broadcast(bc[0:64, :],
                                                  rec2[0:1, a, :])
                    bcs.append(bc)

                def u_norm(a):
                    nc.vector.tensor_tensor(
                        ot[p][64 * a:64 * a + 64, qi * TQ:(qi + 1) * TQ],
                        srcs[a][0:64, :], bcs[a][0:64, :], MULT)

                units = [(50, lambda: u_norm(0), None),
                         (50, lambda: u_norm(1), None)]
                return units

            obs2 = {}

            def po_units(ts_list, act_ob=False):
                def emit_po(ts, dt):
                    po = ps.tile([128, 512], F32,
                                 tag=("sp" if act_ob and dt == 0 else "pq"),
                                 name=f"po_{ts}_{dt}")
                    for ft in range(2):
                        nc.tensor.matmul(
                            po[:], ot[ft][:, ts * 128:(ts + 1) * 128],
                            wout[:, ft, dt * 512:(dt + 1) * 512],
                            start=(ft == 0), stop=(ft == 1))
                    if act_ob:
                        # tail: one [128, 1024] DMA per token tile, staged by
                        # the now-idle Act engine + DVE, drained through the
                        # Act HWDGE and Pool SWDGE queues
                        if dt == 0:
                            obs2[ts] = nrm_p.tile([128, 2, 512], BF16,
                                                  tag="ob0", name=f"obt_{ts}")
                            nc.scalar.activation(obs2[ts][:, 0, :], po[:],
                                                 COPY)
                        else:
                            # tail: Act is idle after the last exp; stage the
                            # second half there too, keeping DVE free for the
                            # piecewise normalize chain
                            nc.scalar.activation(obs2[ts][:, 1, :], po[:],
                                                 COPY)
                            eng = (nc.sync if ts % 2 == 1 else nc.gpsimd)
                            eng.dma_start(
                                out_ap[ts * 128:(ts + 1) * 128, :],
                                obs2[ts][:].rearrange("p a b -> p (a b)"))
                        return
                    ob = nrm_p.tile([128, 512], BF16, tag=f"ob{dt}",
                                    name=f"ob_{ts}_{dt}")
                    nc.vector.tensor_copy(ob[:], po[:])
                    nc.sync.dma_start(
                        out_ap[ts * 128:(ts + 1) * 128,
                               dt * 512:(dt + 1) * 512],
                        ob[:])
                return [
                    (426, (lambda ts=ts, dt=dt: emit_po(ts, dt)), None)
                    for ts in ts_list for dt in range(2)
                ]

            def po_fillers(qi):
                return po_units(range(4 * qi, 4 * qi + 4))

            # ---- Pipeline over 512-token query blocks ----
            # Block 0's Q/K columns: all four 128-col blocks accumulate in
            # parallel (pq banks + the idle av banks) so the PE chews each
            # wqk/xh0 chunk as it lands. Pair 0's evacuations go first (Act +
            # DVE in parallel) so its scores/exp start as early as possible;
            # pair 1's follow, and block-0 V production rides the filler
            # queue between the first AV matmuls.
            pqs = [ps.tile([128, TQ], F32, tag="pq", name=f"pq0_{fb}")
                   for fb in range(2)]
            pqs += [ps_av.tile([128, TQ], F32, tag=f"av{fb - 2}",
                               name=f"pq0_{fb}") for fb in range(2, 4)]
            for j in range(4):
                s = slice(2 * j, 2 * j + 2)
                for fb in range(4):
                    nc.tensor.matmul(pqs[fb][:],
                                     wqk[:, s, fb * 128:(fb + 1) * 128],
                                     xh0[:, s, :], start=(j == 0),
                                     stop=(j == 3), perf_mode=DR,
                                     skip_group_check=True)
            evac_fb(0, 0, pqs[0], "dve")
            evac_fb(0, 2, pqs[2], "act")
            pre0 = {0: emit_scores(0, 0, 0), 1: emit_scores(0, 0, 1)}
            evac_fb(0, 1, pqs[1], "dve")
            evac_fb(0, 3, pqs[3], "act")
            carry = []                  # deferred normalize units
            # Per-block filler tranches, balanced against each block's Act
            # (exp) load; out-projections run two blocks late. Leftover
            # fillers carry across blocks instead of draining at block ends,
            # so the PE keeps feeding scores while Act is busy.
            v0 = v_units(0)
            tranche1 = {
                0: (v0[:2] + q_units(1) + k_units(1) + v0[2:] + v_units(1)
                    + q_units(2) + k_units(2)),
                1: v_units(2) + q_units(3) + k_units(3) + po_fillers(0),
                2: v_units(3) + po_fillers(1),
                3: po_fillers(2),
            }
            fl = deque()
            for tt in range(NQT):
                fl.extend(carry)
                carry = []
                fl.extend(tranche1[tt])
                n0 = emit_c(0, tt, fl, pre_sc=(pre0 if tt == 0 else None))
                for u in reversed(n0):  # pair 0's normalize -> front of queue
                    fl.appendleft(u)
                carry = emit_c(1, tt, fl, defer=(tt < NQT - 1))
            norm_piece = carry      # piecewise normalizer of the last pair
            carry = []
            while fl:
                fl.popleft()[1]()
            tail_po = po_units(range(4 * (NQT - 1), 4 * NQT), act_ob=True)
            for j in range(4):      # norm piece j, then po(12+j) dt0+dt1
                norm_piece(j)
                tail_po[2 * j][1]()
                tail_po[2 * j + 1][1]()


_CACHE = {}


def _program():
    if "nc" in _CACHE:
        return _CACHE["nc"]
    nc = bacc.Bacc("TRN2", target_bir_lowering=False, debug=False)
    ins = {
        "xh": nc.dram_tensor("xh", [D, T], E4M3, kind="ExternalInput").ap(),
        "xl": nc.dram_tensor("xl", [D, T], E4M3, kind="ExternalInput").ap(),
        "wqk": nc.dram_tensor("wqk", [D, 512], E4M3,
                              kind="ExternalInput").ap(),
        "bqk": nc.dram_tensor("bqk", [128, 4], F32, kind="ExternalInput").ap(),
        "wvh": nc.dram_tensor("wvh", [D, 256], E4M3,
                              kind="ExternalInput").ap(),
        "wvl": nc.dram_tensor("wvl", [D, 256], E4M3,
                              kind="ExternalInput").ap(),
        "wout": nc.dram_tensor("wout", [128, 2, 1024], BF16,
                               kind="ExternalInput").ap(),
        "mask8": nc.dram_tensor("mask8", [128, 896], E4M3,
                                kind="ExternalInput").ap(),
        "maskb": nc.dram_tensor("maskb", [128, 896], BF16,
                                kind="ExternalInput").ap(),
    }
    out = nc.dram_tensor("out", [T, D], BF16, kind="ExternalOutput").ap()
    with tile.TileContext(nc) as tc:
        _build_mha(tc, out, ins)
    nc.compile()
    _CACHE["nc"] = nc
    return nc


def _in_maps(x, Wqkv, bqkv, Wout):
    bf16 = ml_dtypes.bfloat16
    e4m3 = ml_dtypes.float8_e4m3
    x = np.asarray(x, dtype=np.float32)
    Wqkv = np.asarray(Wqkv, dtype=np.float32)
    bqkv = np.asarray(bqkv, dtype=np.float32)
    Wout = np.asarray(Wout, dtype=np.float32)
    scale = np.float32(1.0 / np.sqrt(HD))
    mask_f = (np.arange(128)[:, None] <= np.arange(896)[None, :] - 384
              ).astype(np.float32)
    maps = []
    for c in range(NCORES):
        b, hg = c // 4, c % 4
        hs = [4 * hg + i for i in range(HL)]
        q_cols = np.concatenate([Wqkv[:, h * HD:(h + 1) * HD] for h in hs],
                                axis=1)
        k_cols = np.concatenate(
            [Wqkv[:, D + h * HD:D + (h + 1) * HD] for h in hs], axis=1)
        v_cols = np.concatenate(
            [Wqkv[:, 2 * D + h * HD:2 * D + (h + 1) * HD] for h in hs], axis=1)
        bq = np.concatenate([bqkv[h * HD:(h + 1) * HD] for h in hs])
        bk = np.concatenate([bqkv[D + h * HD:D + (h + 1) * HD] for h in hs])
        wqk = np.concatenate([q_cols * scale, k_cols], axis=1) * WS
        bqk = (np.concatenate([bq * scale, bk]) * WS).reshape(4, 128).T
        wv = v_cols * WS
        wvh = wv.astype(e4m3)
        wvl = (wv - wvh.astype(np.float32)).astype(e4m3)
        wo = np.concatenate([Wout[h * HD:(h + 1) * HD, :] for h in hs], axis=0)
        wo = np.ascontiguousarray(
            wo.reshape(2, 128, D).transpose(1, 0, 2)).astype(bf16)
        xt = np.ascontiguousarray(x[b].T)
        xhv = xt.astype(e4m3)
        xlv = (xt - xhv.astype(np.float32)).astype(e4m3)
        maps.append({
            "xh": xhv,
            "xl": xlv,
            "wqk": np.ascontiguousarray(wqk).astype(e4m3),
            "bqk": np.ascontiguousarray(bqk.astype(np.float32)),
            "wvh": np.ascontiguousarray(wvh),
            "wvl": np.ascontiguousarray(wvl),
            "wout": wo,
            "mask8": mask_f.astype(e4m3),
            "maskb": mask_f.astype(bf16),
        })
    return maps


def kernel(x, Wqkv, bqkv, Wout, bout):
    global LAST_RESULTS
    nc = _program()
    maps = _in_maps(x, Wqkv, bqkv, Wout)
    res = run_bass_kernel_spmd(nc, maps, list(range(NCORES)))
    LAST_RESULTS = res
    bout = np.asarray(bout, dtype=np.float32)
    bv_full = np.asarray(bqkv, dtype=np.float32)[2 * D:3 * D]
    bout_eff = bout + bv_full @ np.asarray(Wout, dtype=np.float32)
    out = np.empty((B, T, D), dtype=np.float32)
    inv = np.float32(1.0 / WS)
    for b in range(B):
        acc = res.results[4 * b]["out"].astype(np.float32)
        for hg in range(1, 4):
            acc = acc + res.results[4 * b + hg]["out"]
        out[b] = acc * inv + bout_eff[None, :]
    return out


# revision 47
# speedup vs baseline: 1.0080x; 1.0080x over previous
"""Masked multi-head attention on 8 Trainium2 NeuronCores (fp8 DoubleRow).

Sharding: core c = (b, hg) with b = c // 4, hg = c % 4. Each core computes the
full attention block for batch b restricted to heads [4*hg, 4*hg+4), including
its slice of the QKV projection and of the output projection. The host sums the
4 tensor-parallel partial outputs per batch (dividing out the fp8 weight
pre-scale) and adds the output bias (which also absorbs the V bias term).

Shapes are hardcoded for B=2, T=2048, D=1024, H=16, Hd=64.

Precision plan (validated against the reference in numpy emulation):
- QK projection: x and Wqk in e4m3 (x16 weight pre-scale), DoubleRow over
  dt-chunk pairs -> 4x PE throughput. Q/K fp8 noise largely washes out in
  softmax normalization.
- V projection: both sides residual-split (hi+lo e4m3), 3 DoubleRow terms
  (hi*hi + hi*lo + lo*hi) -> 1.33x, near-bf16 accuracy.
- Scores: kqt holds (K|0) in the two DoubleRow weight slots; moving is the
  broadcast (Q|Q) -> per-head score tile in half the cycles.
- exp on Act with scale=1/256 folding out the x16 Q and K pre-scales; output
  e4m3 for middle query blocks, bf16 for block 0 (few-key softmax rows where
  fp8 P noise is largest) and block 3 (PE/Act balance).
- AV: query blocks 1-2 use DoubleRow (Vhi|Vlo)(P|P) -> V-side exact, P single
  e4m3; blocks 0 and 3 run bf16 AV with a bf16 V copy. Softmax denominators
  ride a 65th ones-column (hi=1, lo=0).
- Out-projection: bf16 throughout (splitting O costs more elementwise work
  than the PE it saves).

Schedule follows the bf16 baseline: host pre-transposes x (zero PE
transposes), biases fused into PSUM evacuations, denominators via the ones
column, normalize deferred into the next pair's filler stream with a
Pool-issued partition broadcast, scores running k-tiles ahead of AV, and a
cost-weighted filler queue (QKV/V/out-proj work) keeping the PE continuously
busy so its p-state never drops. Block 0's Q/K accumulates 4-ways in parallel
behind a PE warmup chain; block-0 PSUM evacuations ride the Act engine while
it is still idle. The tail staggers output staging across Act + DVE and
drains the last DMAs through both the Act HWDGE and Pool SWDGE queues.
"""

import numpy as np
from contextlib import ExitStack
from collections import deque

import ml_dtypes

import concourse.bass as bass
import concourse.bacc as bacc
import concourse.mybir as mybir
import concourse.tile as tile
from concourse.bass_utils import run_bass_kernel_spmd

B, T, D = 2, 2048, 1024
H, HD = 16, 64
HL = 4               # heads per core
NCORES = 8
TQ = 512             # query tile (matmul moving free dim)
TK = 128             # key tile
NQT = T // TQ        # 4
NKT = T // TK        # 16
NDT = D // 128       # 8

F32 = mybir.dt.float32
BF16 = mybir.dt.bfloat16
E4M3 = mybir.dt.float8e4
DR = mybir.MatmulPerfMode.DoubleRow
EXP = mybir.ActivationFunctionType.Exp
COPY = mybir.ActivationFunctionType.Copy
IDENT = mybir.ActivationFunctionType.Identity
MULT = mybir.AluOpType.mult
ADD = mybir.AluOpType.add
SUB = mybir.AluOpType.subtract

WS = 16.0            # fp8 weight pre-scale
ESC = 1.0 / (WS * WS)  # exp reads scores scaled by 16(Q) * 16(K)

LAST_RESULTS = None  # BassKernelResults of the most recent run (for test.py)

import os as _os
_B0 = float(_os.environ.get("KBONUS0", "0"))
_B1 = float(_os.environ.get("KBONUS1", "0"))
_LA = int(_os.environ.get("KLOOKAHEAD", "3"))
_PACE = float(_os.environ.get("KPACE", "0.82"))
_PTB = int(_os.environ.get("KPTBUFS", "9"))

# query blocks whose AV runs bf16 (P stays bf16 there)
BF16_AV_QI = (0, 3)


def _build_mha(tc, out_ap, in_aps):
    nc = tc.nc
    xh_d = in_aps["xh"]        # [D, T]  e4m3  fp8(x^T)
    xl_d = in_aps["xl"]        # [D, T]  e4m3  fp8(x^T - xh)
    wqk_d = in_aps["wqk"]      # [D, 512] e4m3 (16*(Qh0..3|Kh0..3), Q pre-scaled)
    bqk_d = in_aps["bqk"]      # [128, 4] f32 (16x biases, col fb per-partition)
    wvh_d = in_aps["wvh"]      # [D, 256] e4m3 hi of 16*Wv
    wvl_d = in_aps["wvl"]      # [D, 256] e4m3 lo
    wout_d = in_aps["wout"]    # [128, 2, 1024] bf16
    m8_d = in_aps["mask8"]     # [128, 896] e4m3
    mb_d = in_aps["maskb"]     # [128, 896] bf16

    with ExitStack() as ctx:
        ctx.enter_context(nc.allow_low_precision(reason="fp8 matmul pipeline"))
        const = ctx.enter_context(tc.tile_pool(name="const", bufs=1))
        big = ctx.enter_context(tc.tile_pool(name="big", bufs=1))

        # Persistent activations.
        # qqt[:, p, t]: Q^T (x16, e4m3); rows 0:64 even head of pair p,
        # 64:128 odd head.
        qqt = big.tile([128, 2, T], E4M3)
        # kqt[:, p, s, t]: s=0 K^T (x16, e4m3), s=1 zeros (DoubleRow dummy)
        kqt = big.tile([128, 2, 2, T], E4M3)
        # vp[:, kt, s, h, 0:64] = hi/lo of 16*V; [..., 64] ones column
        # (hi=1, lo=0). Only k-tiles 0..11 (blocks 1-2 consume them).
        # head slot padded to 80 cols: the dual-fp8 LDWEIGHTS requires the
        # hi/lo slot stride (4*80=320) to be 64-aligned
        vp = big.tile([128, 12, 2, HL, 80], E4M3)
        # vpb: bf16 16*V for the bf16-AV blocks (all 16 k-tiles)
        vpb = big.tile([128, NKT, HL, 65], BF16)
        # ot[p][:, t]: normalized attention output^T (x16), bf16
        ot = [big.tile([128, T], BF16, name=f"ot{p}") for p in range(2)]

        with ExitStack() as pctx:
            # PSUM: sp(2 banks x2) + pq(1 x2) + av0 + av1 = 8 banks
            ps = pctx.enter_context(tc.tile_pool(name="ps", bufs=2,
                                                 space="PSUM"))
            ps_av = pctx.enter_context(tc.tile_pool(name="ps_av", bufs=1,
                                                    space="PSUM"))
            wpool = pctx.enter_context(tc.tile_pool(name="w", bufs=1))
            xt_p = pctx.enter_context(tc.tile_pool(name="xt", bufs=4))
            pt_p = pctx.enter_context(tc.tile_pool(name="ptile", bufs=_PTB))
            nrm_p = pctx.enter_context(tc.tile_pool(name="nrm", bufs=8))

            # PE p-state warmup: tiny matmuls keep the tensor engine busy
            # through its clock ramp while the first DMAs land.
            wsrc = const.tile([1, 64], BF16)
            nc.gpsimd.memset(wsrc[:], 1.0)
            # preload the exp activation table while Act is idle (1.3us)
            wjunk = const.tile([1, 64], BF16)
            nc.scalar.activation(wjunk[:], wsrc[:], EXP)
            warm = ps.tile([1, 64], F32, tag="pq")
            for _ in range(64):
                nc.tensor.matmul(warm[:], wsrc[0:1, 0:1], wsrc[:],
                                 start=True, stop=True)

            # ones columns + the DoubleRow zero slot, built in-place
            nc.gpsimd.memset(vp[:, :, 0, :, 64], 1.0)
            nc.gpsimd.memset(vp[:, :, 1, :, 64], 0.0)
            nc.gpsimd.memset(vpb[:, :, :, 64], 1.0)
            nc.gpsimd.memset(kqt[:, :, 1, :], 0.0)

            xhs = {}
            xls = {}

            def emit_xh_dma(tt):
                xh = xt_p.tile([128, NDT, TQ], E4M3, tag="xh", name=f"xh_{tt}")
                src = xh_d[:, tt * TQ:(tt + 1) * TQ].rearrange(
                    "(o p) t -> p o t", p=128)
                nc.sync.dma_start(xh[:], src)
                xhs[tt] = xh

            def emit_xl_dma(tt, eng=None):
                xl = xt_p.tile([128, NDT, TQ], E4M3, tag="xl", name=f"xl_{tt}")
                src = xl_d[:, tt * TQ:(tt + 1) * TQ].rearrange(
                    "(o p) t -> p o t", p=128)
                (eng or nc.gpsimd).dma_start(xl[:], src)
                xls[tt] = xl

            # DMA order = first-use order. wqk/xh0 stream in dt-quad chunks
            # so the 4-way parallel QK(0) accumulation can start on the first
            # chunk; side inputs ride the DVE HWDGE queue so they don't wait
            # behind the critical wqk/xh0 stream.
            wqk = wpool.tile([128, NDT, 512], E4M3)
            wqk_src = wqk_d.rearrange("(o p) f -> p o f", p=128)
            xh0 = xt_p.tile([128, NDT, TQ], E4M3, tag="xh", name="xh_0")
            xh0_src = xh_d[:, 0:TQ].rearrange("(o p) t -> p o t", p=128)
            xhs[0] = xh0
            # pair-0's K/Q weight columns ride the first DMA so its score
            # pipeline (and the Act exp stream) starts as early as possible
            for dh in range(2):
                s = slice(4 * dh, 4 * dh + 4)
                nc.sync.dma_start(wqk[:, s, :], wqk_src[:, s, :])
                nc.sync.dma_start(xh0[:, s, :], xh0_src[:, s, :])
            bqk = const.tile([128, 4], F32)
            nc.sync.dma_start(bqk[:], bqk_d)
            maskb = const.tile([128, 896], BF16)
            nc.sync.dma_start(maskb[:], mb_d)
            wvh = wpool.tile([128, NDT, 256], E4M3)
            nc.sync.dma_start(wvh[:], wvh_d.rearrange("(o p) f -> p o f",
                                                        p=128))
            wvl = wpool.tile([128, NDT, 256], E4M3)
            nc.sync.dma_start(wvl[:], wvl_d.rearrange("(o p) f -> p o f",
                                                        p=128))
            emit_xl_dma(0, nc.sync)
            emit_xh_dma(1)
            mask8 = const.tile([128, 896], E4M3)
            nc.gpsimd.dma_start(mask8[:], m8_d)
            wout = const.tile([128, 2, 1024], BF16)
            nc.gpsimd.dma_start(wout[:], wout_d)
            emit_xh_dma(2)
            emit_xl_dma(1)
            emit_xh_dma(3)
            emit_xl_dma(2)
            emit_xl_dma(3)

            def qk_matmuls(pq, fb, xh, skip=False):
                for j in range(4):
                    s = slice(2 * j, 2 * j + 2)
                    nc.tensor.matmul(pq[:], wqk[:, s, fb * 128:(fb + 1) * 128],
                                     xh[:, s, :], start=(j == 0),
                                     stop=(j == 3), perf_mode=DR,
                                     skip_group_check=skip)

            def evac_fb(tt, fb, pq, eng):
                """PSUM -> qqt/kqt with the (x16) bias fused; eng = DVE or Act
                (Act only while it is still idle, pre-exp)."""
                cols = slice(tt * TQ, (tt + 1) * TQ)
                dst = (qqt[:, fb, cols] if fb < 2
                       else kqt[:, fb - 2, 0, cols])
                if eng == "act":
                    nc.scalar.activation(dst, pq[:], IDENT,
                                         bias=bqk[:, fb:fb + 1])
                else:
                    nc.vector.tensor_scalar(out=dst, in0=pq[:],
                                            scalar1=bqk[:, fb:fb + 1],
                                            scalar2=None, op0=ADD)

            def emit_fb(tt, fb):
                """One 128-col block of Q^T/K^T via fp8 DoubleRow. Q
                evacuations ride Act (their consumers stall on the DVE
                queue at block starts); K evacuations stay on DVE."""
                pq = ps.tile([128, TQ], F32, tag="pq", name=f"pq_{tt}_{fb}")
                qk_matmuls(pq, fb, xhs[tt])
                evac_fb(tt, fb, pq, "dve")

            def emit_v(tt, ts):
                """V rows for one 128-token tile: both-split fp8 DoubleRow
                (hi*hi + hi*lo + lo*hi), evacuated to vp hi/lo + bf16 vpb."""
                xh, xl = xhs[tt], xls[tt]
                kt = tt * 4 + ts
                pv = ps.tile([128, 512], F32, tag="pq", name=f"pv_{tt}_{ts}")
                tok = slice(ts * 128, (ts + 1) * 128)
                first = True
                for wv, xx in ((wvh, xh), (wvl, xh), (wvh, xl)):
                    for j in range(4):
                        s = slice(2 * j, 2 * j + 2)
                        nc.tensor.matmul(pv[:, 0:256], xx[:, s, tok],
                                         wv[:, s, :], start=first,
                                         stop=(wv is wvh and xx is xl
                                               and j == 3), perf_mode=DR)
                        first = False
                src = pv[:, 0:256].rearrange("p (h e) -> p h e", e=HD)
                if kt < 12:
                    nc.vector.tensor_copy(vp[:, kt, 0, :, 0:64], src)
                    nc.vector.scalar_tensor_tensor(
                        vp[:, kt, 1, :, 0:64], src, 0.0,
                        vp[:, kt, 0, :, 0:64], ADD, SUB)
                if kt < 4 or kt >= 12:
                    # needed early (block 0) or without a vp slot (kt>=12):
                    # direct PSUM copy on DVE
                    nc.vector.tensor_copy(vpb[:, kt, :, 0:64], src)
                else:
                    # rebuild bf16 V from the fp8 hi/lo on the idle Pool
                    # engine, keeping the DVE queue short
                    nc.gpsimd.tensor_tensor(vpb[:, kt, :, 0:64],
                                            vp[:, kt, 0, :, 0:64],
                                            vp[:, kt, 1, :, 0:64], ADD)

            # (PE-ns cost, closure, deadline) triples for the filler queue.
            # deadline = (qi, kt) before which the unit MUST have emitted
            # (tile-framework program order): budget pacing may defer units,
            # the deadline pop may not.
            def q_units(tt):
                return [(427, (lambda tt=tt, fb=fb: emit_fb(tt, fb)),
                         (tt, -1)) for fb in range(2)]

            def k_units(tt):
                return [(427, (lambda tt=tt, fb=fb: emit_fb(tt, fb)),
                         (tt, max(0, 4 * tt - 4))) for fb in range(2, 4)]

            def v_units(tt):
                return [(640, (lambda tt=tt, ts=ts: emit_v(tt, ts)),
                         (tt, 4 * tt + ts)) for ts in range(4)]

            def pop_due(fillers, qi, kt):
                due = [u for u in fillers if u[2] is not None
                       and u[2] <= (qi, kt)]
                if due:
                    rest = [u for u in fillers if u[2] is None
                            or u[2] > (qi, kt)]
                    fillers.clear()
                    fillers.extend(rest)
                    for _, f, _d in due:
                        f()

            def emit_scores(p, qi, kt):
                """Score tile for both heads of pair p via fp8 DoubleRow with
                the (K|0) weight slots and broadcast (Q|Q) moving; exp + mask.
                bf16 pt for the bf16-AV blocks, e4m3 otherwise."""
                lo = 128 * (kt - 4 * qi)     # causal start, local cols
                c_lo = max(lo, 0)
                w = TQ
                bavq = qi in BF16_AV_QI
                sp = ps.tile([128, 2, TQ], F32, tag="sp",
                             name=f"sp_{p}_{qi}_{kt}")
                for a in range(2):
                    rows = slice(64 * a, 64 * a + 64)
                    mv = qqt[rows, p,
                             qi * TQ + c_lo:(qi + 1) * TQ][:, None, :]
                    nc.tensor.matmul(
                        sp[:, a, c_lo:w],
                        kqt[rows, p, :, kt * TK:(kt + 1) * TK],
                        mv.broadcast_to([64, 2, w - c_lo]),
                        start=True, stop=True, perf_mode=DR)
                pt = pt_p.tile([128, 2, TQ], BF16 if bavq else E4M3,
                               tag=("ptb" if bavq else "pt8"),
                               name=f"pt_{p}_{qi}_{kt}")
                nc.scalar.activation(pt[:, :, c_lo:w], sp[:, :, c_lo:w], EXP,
                                     scale=ESC)
                if -128 < lo < w:
                    c0 = 384 - 128 * (kt - 4 * qi)
                    m_lo, m_hi = c_lo, min(lo + 128, w)
                    msk = maskb if bavq else mask8
                    nc.gpsimd.tensor_tensor(
                        pt[:, :, m_lo:m_hi], pt[:, :, m_lo:m_hi],
                        msk[:, None, c0 + m_lo:c0 + m_hi].broadcast_to(
                            [128, 2, m_hi - m_lo]), MULT)
                return pt, c_lo

            def emit_c(p, qi, fillers, defer=True, pre_sc=None):
                """AV block for pair p. Returns deferred normalize units."""
                w = TQ
                bavq = qi in BF16_AV_QI
                av0 = ps_av.tile([128, TQ], F32, tag="av0",
                                 name=f"av0_{p}_{qi}")
                av1 = ps_av.tile([128, TQ], F32, tag="av1",
                                 name=f"av1_{p}_{qi}")
                nkt = 4 * qi + 4
                pop_due(fillers, qi, -1)
                sc = dict(pre_sc) if pre_sc else {0: emit_scores(p, qi, 0)}
                # flush the previous pair's deferred normalize first (their
                # multiplies release the single-buffered av banks), plus
                # filler to cover the first exp's latency; pair-1 starts also
                # force the next block's Q/K projections through
                for _ in range(min(2, len(fillers))):
                    fillers.popleft()[1]()
                for _k in (1, 2):
                    if _k < nkt and _k not in sc:
                        sc[_k] = emit_scores(p, qi, _k)
                debt = 0.0
                for kt in range(nkt):
                    if kt + _LA < nkt and kt + _LA not in sc:
                        sc[kt + _LA] = emit_scores(p, qi, kt + _LA)
                    # pace filler popping against this k-tile's Act slack:
                    # exp time minus the PE's own score+AV time, so fillers
                    # never crowd out the scores Act is waiting for
                    wc = w - max(128 * (kt - 4 * qi), 0)
                    act_step = (1.667 * wc + 175.0) * _PACE
                    pe_step = 0.4167 * wc * (3.0 if bavq else 2.0)
                    debt += max(act_step - pe_step, 0.0) + (_B0, _B1,
                                                           0.0, 0.0)[qi]
                    pop_due(fillers, qi, kt)
                    while fillers and debt >= fillers[0][0]:
                        c, f, _d = fillers.popleft()
                        f()
                        debt -= c
                    if kt not in sc:
                        sc[kt] = emit_scores(p, qi, kt)
                    pt, c_lo = sc.pop(kt)
                    # av rows 0:64 = O^T (x16), row 64 = softmax denominator
                    for a, av in ((0, av0), (1, av1)):
                        if bavq:
                            nc.tensor.matmul(
                                av[0:65, c_lo:w], vpb[:, kt, 2 * p + a, :],
                                pt[:, a, c_lo:w],
                                start=(kt == 0), stop=(kt == nkt - 1),
                                skip_group_check=True)
                        else:
                            mv = pt[:, a, c_lo:w][:, None, :]
                            nc.tensor.matmul(
                                av[0:65, c_lo:w],
                                vp[:, kt, :, 2 * p + a, 0:65],
                                mv.broadcast_to([128, 2, w - c_lo]),
                                start=(kt == 0), stop=(kt == nkt - 1),
                                perf_mode=DR, skip_group_check=True)
                srcs = [av0, av1]
                if not defer:
                    # last pair: normalize in 128-col pieces so the tail
                    # out-projections pipeline with the normalize chain
                    # (the caller emits po(ts) right after piece ts-12)
                    rt = nrm_p.tile([1, 2, TQ], BF16, tag="rec",
                                    name="rect")
                    bct = [nrm_p.tile([64, TQ], BF16, tag=f"bc{a}",
                                      name=f"bct_{a}") for a in range(2)]

                    def norm_piece(j):
                        cs = slice(j * 128, (j + 1) * 128)
                        ocs = slice(qi * TQ + j * 128, qi * TQ + (j + 1) * 128)
                        for a in range(2):
                            nc.vector.reciprocal(rt[0:1, a, cs],
                                                 srcs[a][64:65, cs])
                            nc.gpsimd.partition_broadcast(bct[a][0:64, cs],
                                                          rt[0:1, a, cs])
                            nc.vector.tensor_tensor(
                                ot[p][64 * a:64 * a + 64, ocs],
                                srcs[a][0:64, cs], bct[a][0:64, cs], MULT)
                    return norm_piece
                # Both reciprocals land on partition 0 of one tile; the
                # multiplies are deferred into the next pair's filler stream.
                rec2 = nrm_p.tile([1, 2, TQ], BF16, tag="rec",
                                  name=f"rec_{p}_{qi}")
                nc.vector.reciprocal(rec2[0:1, 0, :], srcs[0][64:65, :])
                nc.vector.reciprocal(rec2[0:1, 1, :], srcs[1][64:65, :])
                bcs = []
                for a in range(2):
                    bc = nrm_p.tile([64, TQ], BF16, tag=f"bc{a}",
                                    name=f"bc_{p}_{qi}_{a}")
                    nc.gpsimd.partition_broadcast(bc[0:64, :],
                                                  rec2[0:1, a, :])
                    bcs.append(bc)

                def u_norm(a):
                    nc.vector.tensor_tensor(
                        ot[p][64 * a:64 * a + 64, qi * TQ:(qi + 1) * TQ],
                        srcs[a][0:64, :], bcs[a][0:64, :], MULT)

                units = [(50, lambda: u_norm(0), None),
                         (50, lambda: u_norm(1), None)]
                return units

            obs2 = {}

            def po_units(ts_list, act_ob=False):
                def emit_po(ts, dt):
                    po = ps.tile([128, 512], F32,
                                 tag=("sp" if act_ob and dt == 0 else "pq"),
                                 name=f"po_{ts}_{dt}")
                    for ft in range(2):
                        nc.tensor.matmul(
                            po[:], ot[ft][:, ts * 128:(ts + 1) * 128],
                            wout[:, ft, dt * 512:(dt + 1) * 512],
                            start=(ft == 0), stop=(ft == 1))
                    if act_ob:
                        # tail: one [128, 1024] DMA per token tile, staged by
                        # the now-idle Act engine + DVE, drained through the
                        # Act HWDGE and Pool SWDGE queues
                        if dt == 0:
                            obs2[ts] = nrm_p.tile([128, 2, 512], BF16,
                                                  tag="ob0", name=f"obt_{ts}")
                            nc.scalar.activation(obs2[ts][:, 0, :], po[:],
                                                 COPY)
                        else:
                            # tail: Act is idle after the last exp; stage the
                            # second half there too, keeping DVE free for the
                            # piecewise normalize chain
                            nc.scalar.activation(obs2[ts][:, 1, :], po[:],
                                                 COPY)
                            eng = (nc.sync if ts % 2 == 1 else nc.gpsimd)
                            eng.dma_start(
                                out_ap[ts * 128:(ts + 1) * 128, :],
                                obs2[ts][:].rearrange("p a b -> p (a b)"))
                        return
                    ob = nrm_p.tile([128, 512], BF16, tag=f"ob{dt}",
                                    name=f"ob_{ts}_{dt}")
                    nc.vector.tensor_copy(ob[:], po[:])
                    nc.sync.dma_start(
                        out_ap[ts * 128:(ts + 1) * 128,
                               dt * 512:(dt + 1) * 512],
                        ob[:])
                return [
                    (426, (lambda ts=ts, dt=dt: emit_po(ts, dt)), None)
                    for ts in ts_list for dt in range(2)
                ]

            def po_fillers(qi):
                return po_units(range(4 * qi, 4 * qi + 4))

            # ---- Pipeline over 512-token query blocks ----
            # Block 0's Q/K columns: all four 128-col blocks accumulate in
            # parallel (pq banks + the idle av banks) so the PE chews each
            # wqk/xh0 chunk as it lands. Pair 0's evacuations go first (Act +
            # DVE in parallel) so its scores/exp start as early as possible;
            # pair 1's follow, and block-0 V production rides the filler
            # queue between the first AV matmuls.
            pqs = [ps.tile([128, TQ], F32, tag="pq", name=f"pq0_{fb}")
                   for fb in range(2)]
            pqs += [ps_av.tile([128, TQ], F32, tag=f"av{fb - 2}",
                               name=f"pq0_{fb}") for fb in range(2, 4)]
            for j in range(4):
                s = slice(2 * j, 2 * j + 2)
                for fb in range(4):
                    nc.tensor.matmul(pqs[fb][:],
                                     wqk[:, s, fb * 128:(fb + 1) * 128],
                                     xh0[:, s, :], start=(j == 0),
                                     stop=(j == 3), perf_mode=DR,
                                     skip_group_check=True)
            evac_fb(0, 0, pqs[0], "dve")
            evac_fb(0, 2, pqs[2], "act")
            pre0 = {0: emit_scores(0, 0, 0), 1: emit_scores(0, 0, 1)}
            evac_fb(0, 1, pqs[1], "dve")
            evac_fb(0, 3, pqs[3], "act")
            carry = []                  # deferred normalize units
            # Per-block filler tranches, balanced against each block's Act
            # (exp) load; out-projections run two blocks late. Leftover
            # fillers carry across blocks instead of draining at block ends,
            # so the PE keeps feeding scores while Act is busy.
            v0 = v_units(0)
            tranche1 = {
                0: (v0[:2] + q_units(1) + k_units(1) + v0[2:] + v_units(1)
                    + q_units(2) + k_units(2)),
                1: v_units(2) + q_units(3) + k_units(3) + po_fillers(0),
                2: v_units(3) + po_fillers(1),
                3: po_fillers(2),
            }
            fl = deque()
            for tt in range(NQT):
                fl.extend(carry)
                carry = []
                fl.extend(tranche1[tt])
                n0 = emit_c(0, tt, fl, pre_sc=(pre0 if tt == 0 else None))
                for u in reversed(n0):  # pair 0's normalize -> front of queue
                    fl.appendleft(u)
                carry = emit_c(1, tt, fl, defer=(tt < NQT - 1))
            norm_piece = carry      # piecewise normalizer of the last pair
            carry = []
            while fl:
                fl.popleft()[1]()
            tail_po = po_units(range(4 * (NQT - 1), 4 * NQT), act_ob=True)
            for j in range(4):      # norm piece j, then po(12+j) dt0+dt1
                norm_piece(j)
                tail_po[2 * j][1]()
                tail_po[2 * j + 1][1]()


_CACHE = {}


def _program():
    if "nc" in _CACHE:
        return _CACHE["nc"]
    nc = bacc.Bacc("TRN2", target_bir_lowering=False, debug=False)
    ins = {
        "xh": nc.dram_tensor("xh", [D, T], E4M3, kind="ExternalInput").ap(),
        "xl": nc.dram_tensor("xl", [D, T], E4M3, kind="ExternalInput").ap(),
        "wqk": nc.dram_tensor("wqk", [D, 512], E4M3,
                              kind="ExternalInput").ap(),
        "bqk": nc.dram_tensor("bqk", [128, 4], F32, kind="ExternalInput").ap(),
        "wvh": nc.dram_tensor("wvh", [D, 256], E4M3,
                              kind="ExternalInput").ap(),
        "wvl": nc.dram_tensor("wvl", [D, 256], E4M3,
                              kind="ExternalInput").ap(),
        "wout": nc.dram_tensor("wout", [128, 2, 1024], BF16,
                               kind="ExternalInput").ap(),
        "mask8": nc.dram_tensor("mask8", [128, 896], E4M3,
                                kind="ExternalInput").ap(),
        "maskb": nc.dram_tensor("maskb", [128, 896], BF16,
                                kind="ExternalInput").ap(),
    }
    out = nc.dram_tensor("out", [T, D], BF16, kind="ExternalOutput").ap()
    with tile.TileContext(nc) as tc:
        _build_mha(tc, out, ins)
    nc.compile()
    _CACHE["nc"] = nc
    return nc


def _in_maps(x, Wqkv, bqkv, Wout):
    bf16 = ml_dtypes.bfloat16
    e4m3 = ml_dtypes.float8_e4m3
    x = np.asarray(x, dtype=np.float32)
    Wqkv = np.asarray(Wqkv, dtype=np.float32)
    bqkv = np.asarray(bqkv, dtype=np.float32)
    Wout = np.asarray(Wout, dtype=np.float32)
    scale = np.float32(1.0 / np.sqrt(HD))
    mask_f = (np.arange(128)[:, None] <= np.arange(896)[None, :] - 384
              ).astype(np.float32)
    maps = []
    for c in range(NCORES):
        b, hg = c // 4, c % 4
        hs = [4 * hg + i for i in range(HL)]
        q_cols = np.concatenate([Wqkv[:, h * HD:(h + 1) * HD] for h in hs],
                                axis=1)
        k_cols = np.concatenate(
            [Wqkv[:, D + h * HD:D + (h + 1) * HD] for h in hs], axis=1)
        v_cols = np.concatenate(
            [Wqkv[:, 2 * D + h * HD:2 * D + (h + 1) * HD] for h in hs], axis=1)
        bq = np.concatenate([bqkv[h * HD:(h + 1) * HD] for h in hs])
        bk = np.concatenate([bqkv[D + h * HD:D + (h + 1) * HD] for h in hs])
        wqk = np.concatenate([q_cols * scale, k_cols], axis=1) * WS
        bqk = (np.concatenate([bq * scale, bk]) * WS).reshape(4, 128).T
        wv = v_cols * WS
        wvh = wv.astype(e4m3)
        wvl = (wv - wvh.astype(np.float32)).astype(e4m3)
        wo = np.concatenate([Wout[h * HD:(h + 1) * HD, :] for h in hs], axis=0)
        wo = np.ascontiguousarray(
            wo.reshape(2, 128, D).transpose(1, 0, 2)).astype(bf16)
        xt = np.ascontiguousarray(x[b].T)
        xhv = xt.astype(e4m3)
        xlv = (xt - xhv.astype(np.float32)).astype(e4m3)
        maps.append({
            "xh": xhv,
            "xl": xlv,
            "wqk": np.ascontiguousarray(wqk).astype(e4m3),
            "bqk": np.ascontiguousarray(bqk.astype(np.float32)),
            "wvh": np.ascontiguousarray(wvh),
            "wvl": np.ascontiguousarray(wvl),
            "wout": wo,
            "mask8": mask_f.astype(e4m3),
            "maskb": mask_f.astype(bf16),
        })
    return maps


def kernel(x, Wqkv, bqkv, Wout, bout):
    global LAST_RESULTS
    nc = _program()
    maps = _in_maps(x, Wqkv, bqkv, Wout)
    res = run_bass_kernel_spmd(nc, maps, list(range(NCORES)))
    LAST_RESULTS = res
    bout = np.asarray(bout, dtype=np.float32)
    bv_full = np.asarray(bqkv, dtype=np.float32)[2 * D:3 * D]
    bout_eff = bout + bv_full @ np.asarray(Wout, dtype=np.float32)
    out = np.empty((B, T, D), dtype=np.float32)
    inv = np.float32(1.0 / WS)
    for b in range(B):
        acc = res.results[4 * b]["out"].astype(np.float32)
        for hg in range(1, 4):
            acc = acc + res.results[4 * b + hg]["out"]
        out[b] = acc * inv + bout_eff[None, :]
    return out
